# revision 22
# baseline (speedup 1.0000x reference)
"""Trainium2 Bass kernel for nn_AttentionBlock (GroupNorm + MHA + proj + residual).

Contract: kernel(**inputs) takes the FULL inputs of reference.setup_inputs()
and returns the FULL (8, 512, 32, 32) output. Internally: data-parallel over
the batch dim across 8 NeuronCores (batch == 8, one image per core); weights
are replicated, so no collectives are needed.

Design notes (v5):
  * All matmul operands are bf16 (1 PE cycle/column); PSUM stays fp32.
    x itself is cast to bf16 on the host: halves the input DMA and doubles
    DVE throughput on the normalize; the residual add keeps fp32 PSUM + f32
    output so the error stays ~1e-3 against the 2e-2 gate.
  * qkv_w / proj_w are transposed and cast to bf16 ON THE HOST and fed
    pre-transposed via DRAM: no PE transposes, no ACT copies.
  * The ACT-engine softmax exp stream is the kernel's spine (64 ops x
    ~1.1us, dtype-independent rate). The attention is a flat 64-slot
    pipeline; per slot: 2 concurrent K=64 score matmuls -> one contiguous
    [128,1024] exp -> 2 AV matmuls trailing one slot (across window
    boundaries too). Leftover qkv matmuls dribble 1-2 per slot as filler.
  * AV accumulators: two 1-bank [65,512] PSUM tiles per (pair, pixel-half)
    window, rotating through 3 pool slots so window w's softmax-normalize
    (DVE reciprocal from PSUM row 64 + gpsimd broadcast + multiply) never
    stalls window w+1. PSUM: pss 2x[128,1024] (4) + pav 3x[65,512] (3) +
    pq [128,512] (1) = 8 banks exactly.
  * proj runs the nh=0 pixel half as filler inside the last window; the
    nh=1 half runs at the tail in a fresh 2-slot PSUM pool (opened after
    the attention pools close) so consecutive chunks ping-pong.
  * HAM: ~14 junk fp32 self-matmuls run back-to-back during the input-DMA
    window so the PE clock is already 2.4 GHz when qkv starts (otherwise
    the whole qkv/v burst runs at 1.2 GHz).
"""

import sys
from contextlib import ExitStack

for _p in ("/opt/trn_rl_repo", "/root/.axon_site/_ro/trn_rl_repo"):
    if _p not in sys.path:
        sys.path.append(_p)

import numpy as np
import ml_dtypes

import concourse.bacc as bacc
import concourse.mybir as mybir
import concourse.tile as tile
from concourse.bass_utils import run_bass_kernel_spmd

F32 = mybir.dt.float32
BF16 = mybir.dt.bfloat16

B, C, HW = 8, 512, 1024
GROUPS, HEADS, DH = 32, 8, 64
EPS = 1e-5
KC = C // 128            # 4 channel chunks of 128
N_CORES = 8
AF = mybir.ActivationFunctionType
OP = mybir.AluOpType


def _group_mats():
    # A[kc][c, g] = 1/16 if channel (128*kc + c) is in group g  (mean weights)
    # E[kc][g, c] = 1.0  if channel (128*kc + c) is in group g  (broadcast)
    A = np.zeros((KC, 128, GROUPS), np.float32)
    E = np.zeros((KC, GROUPS, 128), np.float32)
    gsz = C // GROUPS  # 16
    for kc in range(KC):
        for c in range(128):
            g = (kc * 128 + c) // gsz
            A[kc, c, g] = 1.0 / gsz
            E[kc, g, c] = 1.0
    return A, E


def _build():
    nc = bacc.Bacc()

    x_h = nc.dram_tensor("xb", [C, HW], F32, kind="ExternalInput")
    # host-pretransposed, bf16: wqkT[c, o] covers q (o 0:512) and k (512:1024)
    wqkT_h = nc.dram_tensor("wqkT", [C, 2 * C], BF16, kind="ExternalInput")
    wvT_h = nc.dram_tensor("wvT", [C, C], BF16, kind="ExternalInput")
    pwT_h = nc.dram_tensor("pwT", [C, C], BF16, kind="ExternalInput")
    qkvb_h = nc.dram_tensor("qkv_b", [3 * C], F32, kind="ExternalInput")
    pb_h = nc.dram_tensor("proj_b", [C], F32, kind="ExternalInput")
    gnw_h = nc.dram_tensor("gn_w", [C], F32, kind="ExternalInput")
    gnb_h = nc.dram_tensor("gn_b", [C], F32, kind="ExternalInput")
    y_h = nc.dram_tensor("out", [C, HW], F32, kind="ExternalOutput")

    A_np, E_np = _group_mats()
    A_h = nc.inline_tensor(A_np, name="gn_aggr")
    E_h = nc.inline_tensor(E_np, name="gn_bcast")

    with tile.TileContext(nc) as tc, ExitStack() as ctx:
        per = ctx.enter_context(tc.tile_pool(name="per", bufs=1))
        gwork = ctx.enter_context(tc.tile_pool(name="gwork", bufs=2))
        expp = ctx.enter_context(tc.tile_pool(name="expp", bufs=6))
        recp = ctx.enter_context(tc.tile_pool(name="recp", bufs=2))
        outp = ctx.enter_context(tc.tile_pool(name="outp", bufs=2))

        # ---------- persistent tiles ----------
        XC = [per.tile([128, HW], F32, name=f"XC{i}", tag=f"XC{i}") for i in range(KC)]
        xn = [per.tile([128, HW], BF16, name=f"xn{i}", tag=f"xn{i}") for i in range(KC)]
        wqkT = [per.tile([128, 1024], BF16, name=f"wqkT{i}", tag=f"wqkT{i}") for i in range(KC)]
        wvT = [per.tile([128, 512], BF16, name=f"wvT{i}", tag=f"wvT{i}") for i in range(KC)]
        pwT = [per.tile([128, 512], BF16, name=f"pwT{i}", tag=f"pwT{i}") for i in range(KC)]
        qk = [per.tile([128, HW], BF16, name=f"qk{i}", tag=f"qk{i}") for i in range(8)]
        vt = [per.tile([128, HEADS * (DH + 1)], BF16, name=f"vt{i}", tag=f"vt{i}")
              for i in range(8)]
        att = [per.tile([128, HW], BF16, name=f"att{i}", tag=f"att{i}") for i in range(KC)]

        # consolidated small tensors (one strided DMA each)
        gnwt = per.tile([128, KC], F32, name="gnwt", tag="gnwt")
        gnbt = per.tile([128, KC], F32, name="gnbt", tag="gnbt")
        pbt = per.tile([128, KC], F32, name="pbt", tag="pbt")
        qkbt = per.tile([128, 8], F32, name="qkbt", tag="qkbt")
        AtT = per.tile([128, KC, GROUPS], F32, name="AtT", tag="AtT")
        EtT = per.tile([GROUPS, KC, 128], F32, name="EtT", tag="EtT")
        eps_t = per.tile([GROUPS, 1], F32, name="eps", tag="eps")
        bvb = per.tile([128, 512], F32, name="bvb", tag="bvb")
        ones8 = per.tile([128, HEADS], F32, name="ones8", tag="ones8")
        scr = per.tile([128, 512], F32, name="scr", tag="scr")

        # ---------- input DMAs ----------
        # x on two HWDGE queues (sync + scalar); q/k weights follow on sync;
        # everything else on gpsimd, highest-urgency first.
        for kc in range(2):
            nc.sync.dma_start(out=XC[kc], in_=x_h[kc * 128:(kc + 1) * 128, :])
        for kc in range(2, KC):
            nc.scalar.dma_start(out=XC[kc], in_=x_h[kc * 128:(kc + 1) * 128, :])
        for kc in range(KC):
            nc.sync.dma_start(out=wqkT[kc], in_=wqkT_h[kc * 128:(kc + 1) * 128, :])
        nc.gpsimd.dma_start(out=AtT, in_=A_h.rearrange("k c g -> c k g"))
        nc.gpsimd.dma_start(out=EtT, in_=E_h.rearrange("k g c -> g k c"))
        nc.gpsimd.dma_start(out=gnwt, in_=gnw_h.rearrange("(k p) -> p k", p=128))
        nc.gpsimd.dma_start(out=gnbt, in_=gnb_h.rearrange("(k p) -> p k", p=128))
        nc.gpsimd.dma_start(out=qkbt, in_=qkvb_h[0:1024].rearrange("(k p) -> p k", p=128))
        for kc in range(KC):
            nc.gpsimd.dma_start(out=wvT[kc], in_=wvT_h[kc * 128:(kc + 1) * 128, :])
        nc.gpsimd.dma_start(out=bvb[:],
                            in_=qkvb_h[1024:1536].unsqueeze(0).partition_broadcast(128))
        nc.gpsimd.dma_start(out=pbt, in_=pb_h.rearrange("(k p) -> p k", p=128))
        for kc in range(KC):
            nc.gpsimd.dma_start(out=pwT[kc], in_=pwT_h[kc * 128:(kc + 1) * 128, :])

        nc.vector.memset(eps_t, EPS)
        nc.vector.memset(ones8, 1.0)
        nc.vector.memset(scr, 1e-9)

        # ---------- groupnorm (+ HAM warmup) ----------
        with tc.tile_pool(name="ps_gn", bufs=1, space="PSUM") as ps_gn, \
             tc.tile_pool(name="ps_cb", bufs=2, space="PSUM") as ps_cb, \
             tc.tile_pool(name="ps_wm", bufs=1, space="PSUM") as ps_wm:
            # junk fp32 matmuls, back-to-back through the DMA window, so the
            # PE's HAM clock gate is at 8/8 (2.4 GHz) when real work starts
            wmt = ps_wm.tile([128, 512], F32, name="wmt", tag="wmt")
            for _ in range(12):
                nc.tensor.matmul(wmt[:], scr[:, 0:128], scr[:],
                                 start=True, stop=True)

            gstat = ps_gn.tile([GROUPS, 2], F32, name="gstat", tag="gstat")
            me = []
            for kc in range(KC):
                stats = gwork.tile([128, 2, 6], F32, name="stats", tag="stats")
                xv = XC[kc][:].rearrange("p (s f) -> p s f", f=512)
                for s in range(2):
                    nc.vector.bn_stats(out=stats[:, s, :], in_=xv[:, s, :])
                mv = gwork.tile([128, 2], F32, name="mv", tag="mv")
                nc.vector.bn_aggr(out=mv, in_=stats)
                m = gwork.tile([128, 2], F32, name="me", tag="me")
                nc.vector.tensor_mul(out=m[:, 1:2], in0=mv[:, 0:1], in1=mv[:, 0:1])
                nc.vector.tensor_add(out=m[:, 1:2], in0=m[:, 1:2], in1=mv[:, 1:2])
                nc.vector.tensor_copy(out=m[:, 0:1], in_=mv[:, 0:1])
                me.append(m)

            for kc in range(KC):
                nc.tensor.matmul(gstat[:], AtT[:, kc, :], me[kc][:],
                                 start=(kc == 0), stop=(kc == KC - 1))
            gs = gwork.tile([GROUPS, 2], F32, name="gs", tag="gs")
            nc.vector.tensor_copy(out=gs, in_=gstat)
            var = gwork.tile([GROUPS, 1], F32, name="var", tag="var")
            nc.vector.tensor_mul(out=var, in0=gs[:, 0:1], in1=gs[:, 0:1])
            nc.vector.tensor_sub(out=var, in0=gs[:, 1:2], in1=var)
            srt = gwork.tile([GROUPS, 1], F32, name="srt", tag="srt")
            nc.scalar.activation(out=srt, in_=var, func=AF.Sqrt,
                                 bias=eps_t[:], scale=1.0)
            gmr = gwork.tile([GROUPS, 2], F32, name="gmr", tag="gmr")
            nc.vector.reciprocal(out=gmr[:, 1:2], in_=srt)
            nc.vector.tensor_copy(out=gmr[:, 0:1], in_=gs[:, 0:1])

            for kc in range(KC):
                cb = ps_cb.tile([128, 2], F32, name="cb", tag="cb")
                nc.tensor.matmul(cb[:], EtT[:, kc, :], gmr[:],
                                 start=True, stop=True)
                cbs = gwork.tile([128, 2], F32, name="cbs", tag="cbs")
                nc.vector.tensor_copy(out=cbs, in_=cb)
                sc = gwork.tile([128, 1], F32, name=f"sc{kc}", tag=f"sc{kc}")
                sh = gwork.tile([128, 1], F32, name=f"sh{kc}", tag=f"sh{kc}")
                nc.vector.tensor_mul(out=sc, in0=cbs[:, 1:2], in1=gnwt[:, kc:kc + 1])
                nc.vector.tensor_mul(out=sh, in0=cbs[:, 0:1], in1=sc)
                nc.vector.tensor_sub(out=sh, in0=gnbt[:, kc:kc + 1], in1=sh)
                nc.vector.tensor_scalar(out=xn[kc][:], in0=XC[kc][:],
                                        scalar1=sc[:], scalar2=sh[:],
                                        op0=OP.mult, op1=OP.add)

        # ---------- qkv / attention / proj ----------
        with tc.tile_pool(name="ps_q", bufs=1, space="PSUM") as ps_q:
            def emit_qk_half(oc, nh):
                pq = ps_q.tile([128, 512], F32, name="pq", tag="pq")
                for kc in range(KC):
                    nc.tensor.matmul(
                        pq[:],
                        wqkT[kc][:, oc * 128:(oc + 1) * 128],
                        xn[kc][:, nh * 512:(nh + 1) * 512],
                        start=(kc == 0), stop=(kc == KC - 1),
                        skip_group_check=True)
                    yield
                nc.vector.tensor_scalar(out=qk[oc][:, nh * 512:(nh + 1) * 512],
                                        in0=pq[:], scalar1=qkbt[:, oc:oc + 1],
                                        scalar2=None, op0=OP.add)
                yield

            def emit_vt(jc):
                pv = ps_q.tile([128, 512], F32, name="pv", tag="pq")
                for kc in range(KC):
                    nc.tensor.matmul(pv[:],
                                     xn[kc][:, jc * 128:(jc + 1) * 128],
                                     wvT[kc][:],
                                     start=(kc == 0), stop=(kc == KC - 1))
                    yield
                vt3 = vt[jc][:].rearrange("p (h e) -> p h e", h=HEADS)
                nc.vector.tensor_copy(out=vt3[:, :, DH:DH + 1],
                                      in_=ones8[:].unsqueeze(-1))
                nc.vector.tensor_add(
                    out=vt3[:, :, 0:DH],
                    in0=pv[:].rearrange("p (h d) -> p h d", h=HEADS),
                    in1=bvb[:].rearrange("p (h d) -> p h d", h=HEADS))
                yield

            def emit_proj_half(oc, nh, pool, tag):
                pp = pool.tile([128, 512], F32, name="pp", tag=tag)
                for kc in range(KC):
                    nc.tensor.matmul(
                        pp[:],
                        pwT[kc][:, oc * 128:(oc + 1) * 128],
                        att[kc][:, nh * 512:(nh + 1) * 512],
                        start=(kc == 0), stop=(kc == KC - 1),
                        skip_group_check=True)
                    yield
                ot = outp.tile([128, 512], F32, name="ot", tag="ot")
                nc.vector.scalar_tensor_tensor(
                    out=ot[:], in0=pp[:], scalar=pbt[:, oc:oc + 1],
                    in1=XC[oc][:, nh * 512:(nh + 1) * 512],
                    op0=OP.add, op1=OP.add)
                nc.sync.dma_start(
                    out=y_h[oc * 128:(oc + 1) * 128, nh * 512:(nh + 1) * 512],
                    in_=ot[:])
                yield

            # deadline order: window (pr,0) needs q[pr]-nh0 and all of
            # k[4+pr] (nh1 from jc=4); window (pr,1) needs q[pr]-nh1.
            def qk_stream():
                for oc, nh in ((1, 0), (5, 0), (5, 1), (1, 1),
                               (2, 0), (6, 0), (6, 1), (2, 1),
                               (3, 0), (7, 0), (7, 1), (3, 1)):
                    yield from emit_qk_half(oc, nh)

            def proj0_stream():
                for oc in range(KC):
                    yield from emit_proj_half(oc, 0, ps_q, "pq")
            fill_qk = qk_stream()
            fill_proj = proj0_stream()

            def emit_filler(gen, n):
                for _ in range(n):
                    try:
                        next(gen)
                    except StopIteration:
                        break

            # upfront: q/k for pair 0 and all v tiles (v feeds the very
            # first AVs). A 4-deep scratch pool keeps these 40 matmuls
            # back-to-back (a single PSUM slot serializes each group behind
            # the previous group's DVE read, ~2x slower and HAM-hostile).
            with tc.tile_pool(name="ps_pre", bufs=4, space="PSUM") as ps_pre:
                def pre_qk_half(oc, nh):
                    pq = ps_pre.tile([128, 512], F32, name="ppre", tag="ppre")
                    for kc in range(KC):
                        nc.tensor.matmul(
                            pq[:],
                            wqkT[kc][:, oc * 128:(oc + 1) * 128],
                            xn[kc][:, nh * 512:(nh + 1) * 512],
                            start=(kc == 0), stop=(kc == KC - 1),
                            skip_group_check=True)
                    nc.vector.tensor_scalar(
                        out=qk[oc][:, nh * 512:(nh + 1) * 512],
                        in0=pq[:], scalar1=qkbt[:, oc:oc + 1],
                        scalar2=None, op0=OP.add)

                def pre_vt(jc):
                    pv = ps_pre.tile([128, 512], F32, name="ppre", tag="ppre")
                    for kc in range(KC):
                        nc.tensor.matmul(pv[:],
                                         xn[kc][:, jc * 128:(jc + 1) * 128],
                                         wvT[kc][:],
                                         start=(kc == 0), stop=(kc == KC - 1),
                                         skip_group_check=True)
                    vt3 = vt[jc][:].rearrange("p (h e) -> p h e", h=HEADS)
                    nc.vector.tensor_copy(out=vt3[:, :, DH:DH + 1],
                                          in_=ones8[:].unsqueeze(-1))
                    nc.vector.tensor_add(
                        out=vt3[:, :, 0:DH],
                        in0=pv[:].rearrange("p (h d) -> p h d", h=HEADS),
                        in1=bvb[:].rearrange("p (h d) -> p h d", h=HEADS))

                for nh in range(2):
                    pre_qk_half(0, nh)
                    pre_qk_half(4, nh)
                for jc in range(8):
                    pre_vt(jc)

            with tc.tile_pool(name="ps_s", bufs=2, space="PSUM") as ps_s, \
                 tc.tile_pool(name="ps_av", bufs=3, space="PSUM") as ps_av:

                # flat 64-slot attention pipeline; AV trails by one slot,
                # crossing window boundaries so the exp stream never waits
                # for a window's last AV pair.
                win = {}

                def new_window(w):
                    pav = [ps_av.tile([DH + 1, 512], F32, name=f"pav{t}",
                                      tag="pav") for t in range(2)]
                    win[w] = (pav, [None] * 8)

                def emit_av(w, jc):
                    pav, exs = win[w]
                    pr = w // 2
                    for t in range(2):
                        h = 2 * pr + t
                        nc.tensor.matmul(
                            pav[t][:],
                            vt[jc][:, h * (DH + 1):(h + 1) * (DH + 1)],
                            exs[jc][:, t * 512:(t + 1) * 512],
                            start=(jc == 0), stop=(jc == 7),
                            skip_group_check=True)

                def emit_norm(w):
                    pav, _ = win.pop(w)
                    pr, hf = w // 2, w % 2
                    for t in range(2):
                        dn = recp.tile([1, 512], F32, name=f"den{t}",
                                       tag=f"den{t}")
                        nc.vector.tensor_copy(out=dn[:],
                                              in_=pav[t][DH:DH + 1, :])
                        rc = recp.tile([1, 512], F32, name=f"rec{t}",
                                       tag=f"rec{t}")
                        nc.vector.reciprocal_approx_fast(out=rc[:], in_=dn[:])
                        rb = recp.tile([DH, 512], F32, name=f"rb{t}",
                                       tag=f"rb{t}")
                        nc.gpsimd.partition_broadcast(out_ap=rb[:],
                                                      in_ap=rc[:])
                        nc.vector.tensor_mul(
                            out=att[pr][64 * t:64 * t + DH,
                                        hf * 512:(hf + 1) * 512],
                            in0=pav[t][0:DH, :],
                            in1=rb[:])

                for s in range(64):
                    w, jc = s // 8, s % 8
                    pr, hf = w // 2, w % 2
                    if jc == 0:
                        new_window(w)
                    qt, kt = qk[pr], qk[4 + pr]
                    pss = ps_s.tile([128, HW], F32, name="pss", tag="pss")
                    for t in range(2):
                        nc.tensor.matmul(
                            pss[:, t * 512:(t + 1) * 512],
                            kt[64 * t:64 * t + DH, jc * 128:(jc + 1) * 128],
                            qt[64 * t:64 * t + DH, hf * 512:(hf + 1) * 512],
                            start=True, stop=True)
                    ex = expp.tile([128, HW], BF16, name="expT", tag="expT")
                    if s == 63:
                        for t in range(2):
                            nc.scalar.activation(
                                out=ex[:, t * 512:(t + 1) * 512],
                                in_=pss[:, t * 512:(t + 1) * 512],
                                func=AF.Exp, scale=float(DH) ** -0.5)
                    else:
                        nc.scalar.activation(out=ex[:], in_=pss[:],
                                             func=AF.Exp,
                                             scale=float(DH) ** -0.5)
                    win[w][1][jc] = ex
                    if s > 0:
                        wp, jp = (s - 1) // 8, (s - 1) % 8
                        emit_av(wp, jp)
                        if jp == 7:
                            emit_norm(wp)
                    if w == 7:
                        emit_filler(fill_proj, 3)
                    else:
                        emit_filler(fill_qk, 2 if jc in (0, 4) else 1)
                # last AV pair: emit each head's AV followed right away
                # by its normalize chain so head 0's chain overlaps head 1's
                # matmul
                pav7, exs7 = win.pop(7)
                for t in range(2):
                    nc.tensor.matmul(
                        pav7[t][:],
                        vt[7][:, (6 + t) * (DH + 1):(7 + t) * (DH + 1)],
                        exs7[7][:, t * 512:(t + 1) * 512],
                        start=False, stop=True,
                        skip_group_check=True)
                    dn = recp.tile([1, 512], F32, name=f"den{t}",
                                   tag=f"den{t}")
                    nc.vector.tensor_copy(out=dn[:],
                                          in_=pav7[t][DH:DH + 1, :])
                    rc = recp.tile([1, 512], F32, name=f"rec{t}",
                                   tag=f"rec{t}")
                    nc.vector.reciprocal_approx_fast(out=rc[:], in_=dn[:])
                    rb = recp.tile([DH, 512], F32, name=f"rb{t}",
                                   tag=f"rb{t}")
                    nc.gpsimd.partition_broadcast(out_ap=rb[:], in_ap=rc[:])
                    nc.vector.tensor_mul(
                        out=att[3][64 * t:64 * t + DH, 512:1024],
                        in0=pav7[t][0:DH, :], in1=rb[:])
                emit_filler(fill_qk, 1000)
                emit_filler(fill_proj, 1000)

            # tail: proj nh=1 in a fresh 2-slot pool (attention PSUM freed)
            with tc.tile_pool(name="ps_p2", bufs=2, space="PSUM") as ps_p2:
                for oc in range(KC):
                    for _ in emit_proj_half(oc, 1, ps_p2, "pp2"):
                        pass
    nc.compile()
    return nc


_NC = None


def _get_nc():
    global _NC
    if _NC is None:
        _NC = _build()
    return _NC


def _run(inputs, **kwargs):
    nc = _get_nc()
    x = np.asarray(inputs["x"], dtype=np.float32)
    qkv_w = np.asarray(inputs["qkv_w"], np.float32)
    proj_w = np.asarray(inputs["proj_w"], np.float32)
    shared = {
        "wqkT": np.ascontiguousarray(qkv_w[0:1024].T).astype(ml_dtypes.bfloat16),
        "wvT": np.ascontiguousarray(qkv_w[1024:1536].T).astype(ml_dtypes.bfloat16),
        "pwT": np.ascontiguousarray(proj_w.T).astype(ml_dtypes.bfloat16),
        "qkv_b": np.ascontiguousarray(np.asarray(inputs["qkv_b"], np.float32)),
        "proj_b": np.ascontiguousarray(np.asarray(inputs["proj_b"], np.float32)),
        "gn_w": np.ascontiguousarray(np.asarray(inputs["gn_w"], np.float32)),
        "gn_b": np.ascontiguousarray(np.asarray(inputs["gn_b"], np.float32)),
    }
    xb = x.reshape(B, C, HW)
    in_maps = [dict(shared, xb=np.ascontiguousarray(xb[m])) for m in range(B)]
    res = run_bass_kernel_spmd(nc, in_maps, core_ids=list(range(N_CORES)), **kwargs)
    out = np.stack([res.results[m]["out"] for m in range(B)])
    return out.reshape(B, C, 32, 32).astype(np.float32), res


def kernel(**inputs):
    out, _ = _run(inputs)
    return out
